# revision 1
# baseline (speedup 1.0000x reference)
# Involution2d (K=7) Trainium2 kernel — 8-core SPMD, batch+spatial sharding.
#
# Sharding: 8 cores = (batch b in 0..3) x (H-half in 0..1). Each core computes
# a [128, 32, 64] output block. Per core, on device:
#   1. kernel generation: 1x1 conv (BN folded) -> ReLU -> 1x1 conv -> [49, 2048]
#      per-pixel involution kernels (+ bias, x boundary mask folded in).
#   2. involution: acc[c, p] = sum_o kerm[o, p] * xw[c, p + shift_o]
#      - ker rows broadcast across 128 partitions via K=1 matmuls on TensorE
#      - multiply/accumulate on VectorE; row shifts are free-dim AP offsets
#        into a halo-padded x layout; W-edge wraps are killed by the mask.
import numpy as np

EPS = 1e-5
KK = 7
C = 128
H = 64
W = 64
B = 4
HH = 32            # rows per core
P = HH * W         # 2048 output pixels per core
NIN = 4 + 38 * W + 4   # 2440: 3-row halos + 4-elem guard pads each side
GEN_CHUNK = 512
BC_CHUNK = 1024    # broadcast/psum chunk (2 PSUM banks)

_STATE = {}


def _build():
    import concourse.tile as tile
    from concourse import bacc, mybir

    f32 = mybir.dt.float32
    nc = bacc.Bacc("TRN2", target_bir_lowering=False, debug=False)

    xw_d = nc.dram_tensor("xw", [C, NIN], f32, kind="ExternalInput").ap()
    w1sT_d = nc.dram_tensor("w1sT", [C, 32], f32, kind="ExternalInput").ap()
    b1f_d = nc.dram_tensor("b1f", [32, 1], f32, kind="ExternalInput").ap()
    w2T_d = nc.dram_tensor("w2T", [32, 49], f32, kind="ExternalInput").ap()
    b2f_d = nc.dram_tensor("b2f", [49, 1], f32, kind="ExternalInput").ap()
    mask_d = nc.dram_tensor("maskt", [49, P], f32, kind="ExternalInput").ap()
    out_d = nc.dram_tensor("out", [C, P], f32, kind="ExternalOutput").ap()

    with tile.TileContext(nc) as tc:
        with (
            tc.tile_pool(name="consts", bufs=1) as cpool,
            tc.tile_pool(name="work", bufs=2) as wpool,
            tc.tile_pool(name="pgen", bufs=2, space="PSUM") as pgen,
            tc.tile_pool(name="pbc", bufs=2, space="PSUM") as pbc,
        ):
            x_sb = cpool.tile([C, NIN], f32, tag="x")
            nc.sync.dma_start(x_sb[:], xw_d)
            w1sT = cpool.tile([C, 32], f32, tag="w1")
            nc.sync.dma_start(w1sT[:], w1sT_d)
            b1f = cpool.tile([32, 1], f32, tag="b1")
            nc.sync.dma_start(b1f[:], b1f_d)
            w2T = cpool.tile([32, 49], f32, tag="w2")
            nc.sync.dma_start(w2T[:], w2T_d)
            b2f = cpool.tile([49, 1], f32, tag="b2")
            nc.sync.dma_start(b2f[:], b2f_d)
            mask_sb = cpool.tile([49, P], f32, tag="mask")
            nc.sync.dma_start(mask_sb[:], mask_d)
            ones_sb = cpool.tile([1, C], f32, tag="ones")
            nc.vector.memset(ones_sb[:], 1.0)

            f_sb = cpool.tile([32, P], f32, tag="f")
            kerm_sb = cpool.tile([49, P], f32, tag="kerm")
            acc_sb = cpool.tile([C, P], f32, tag="acc")

            # ---- kernel generation ----
            # x view for the core's own rows: starts 3 halo rows in (+4 guard)
            XOFF = 4 + 3 * W
            for ci in range(P // GEN_CHUNK):
                sl = slice(ci * GEN_CHUNK, (ci + 1) * GEN_CHUNK)
                xsl = slice(XOFF + ci * GEN_CHUNK, XOFF + (ci + 1) * GEN_CHUNK)
                f1 = pgen.tile([32, GEN_CHUNK], f32, tag="f1")
                nc.tensor.matmul(f1[:], w1sT[:], x_sb[:, xsl], start=True, stop=True)
                # f = relu(f1 + b1f)  (ScalarE, per-partition bias)
                nc.scalar.activation(
                    f_sb[:, sl], f1[:], mybir.ActivationFunctionType.Relu,
                    bias=b1f[:],
                )
                k2 = pgen.tile([49, GEN_CHUNK], f32, tag="k2")
                nc.tensor.matmul(k2[:], w2T[:], f_sb[:, sl], start=True, stop=True)
                # kerm = (k2 + b2) * mask  (VectorE fused)
                nc.vector.scalar_tensor_tensor(
                    out=kerm_sb[:, sl], in0=k2[:], scalar=b2f[:],
                    in1=mask_sb[:, sl],
                    op0=mybir.AluOpType.add, op1=mybir.AluOpType.mult,
                )

            # ---- involution accumulate ----
            NB = BC_CHUNK // 512
            for o in range(49):
                ip, jp = divmod(o, 7)
                A = W * ip + jp + 1
                # matmul rhs must start at partition 0 -> DMA ker row o there
                krow = wpool.tile([1, P], f32, tag="krow")
                nc.sync.dma_start(krow[:], kerm_sb[o:o + 1, :])
                for h2 in range(P // BC_CHUNK):
                    bc = pbc.tile([C, BC_CHUNK], f32, tag="bc")
                    base = h2 * BC_CHUNK
                    for nb in range(NB):
                        nc.tensor.matmul(
                            bc[:, nb * 512:(nb + 1) * 512],
                            ones_sb[:],
                            krow[0:1, base + nb * 512: base + (nb + 1) * 512],
                            start=True, stop=True,
                        )
                    xs = x_sb[:, A + base: A + base + BC_CHUNK]
                    osl = slice(base, base + BC_CHUNK)
                    if o == 0:
                        nc.vector.tensor_mul(acc_sb[:, osl], xs, bc[:])
                    else:
                        prod = wpool.tile([C, BC_CHUNK], f32, tag="prod")
                        nc.vector.tensor_mul(prod[:], xs, bc[:])
                        nc.vector.tensor_add(acc_sb[:, osl], acc_sb[:, osl], prod[:])

            nc.sync.dma_start(out_d, acc_sb[:])

    nc.compile()
    return nc


def _get_nc():
    if "nc" not in _STATE:
        _STATE["nc"] = _build()
    return _STATE["nc"]


def _host_prep(x, w1, b1, bn_gamma, bn_beta, bn_mean, bn_var, w2, b2):
    x = np.asarray(x, dtype=np.float32)
    scale = np.asarray(bn_gamma) / np.sqrt(np.asarray(bn_var) + EPS)
    w1s = (np.asarray(w1) * scale[:, None]).astype(np.float32)
    b1f = (np.asarray(b1) * scale + np.asarray(bn_beta)
           - np.asarray(bn_mean) * scale).astype(np.float32)
    w1sT = np.ascontiguousarray(w1s.T)                      # [128, 32]
    w2T = np.ascontiguousarray(np.asarray(w2, np.float32).T)  # [32, 49]
    b1fc = np.ascontiguousarray(b1f[:, None])               # [32, 1]
    b2fc = np.ascontiguousarray(np.asarray(b2, np.float32)[:, None])  # [49, 1]

    # W-edge mask: kerm[o, p] = 0 where w + dj leaves the row
    wcol = np.arange(P, dtype=np.int64) % W
    maskt = np.zeros((49, P), dtype=np.float32)
    for ipp in range(KK):
        for jpp in range(KK):
            dj = jpp - 3
            maskt[ipp * KK + jpp] = ((wcol + dj >= 0) & (wcol + dj < W))
    maskt = np.ascontiguousarray(maskt)

    in_maps = []
    for core in range(8):
        b, half = divmod(core, 2)
        h0 = HH * half
        xw = np.zeros((C, NIN), dtype=np.float32)
        lo = max(0, h0 - 3)
        hi = min(H, h0 + HH + 3)
        # rows [lo, hi) -> xw positions 4 + 64*(row - h0 + 3)
        src = x[b, :, lo:hi, :].reshape(C, -1)
        start = 4 + W * (lo - h0 + 3)
        xw[:, start:start + src.shape[1]] = src
        in_maps.append({
            "xw": xw, "w1sT": w1sT, "b1f": b1fc, "w2T": w2T,
            "b2f": b2fc, "maskt": maskt,
        })
    return in_maps


def run(inputs: dict, trace: bool = False):
    from concourse.bass_utils import run_bass_kernel_spmd

    nc = _get_nc()
    in_maps = _host_prep(**inputs)
    res = run_bass_kernel_spmd(
        nc, in_maps, core_ids=list(range(8)), trace=trace,
    )
    out = np.zeros((B, C, H, W), dtype=np.float32)
    for core in range(8):
        b, half = divmod(core, 2)
        h0 = HH * half
        out[b, :, h0:h0 + HH, :] = res.results[core]["out"].reshape(C, HH, W)
    return out, res


def kernel(**inputs) -> np.ndarray:
    out, _ = run(inputs, trace=False)
    return out



# revision 4
# speedup vs baseline: 1.5928x; 1.5928x over previous
# Involution2d (K=7) Trainium2 kernel — 8-core SPMD, batch+spatial sharding.
#
# Sharding: 8 cores = (batch b in 0..3) x (H-half in 0..1). Each core computes
# a [128, 32, 64] output block.
#
# Per-core pipeline (fp16 data path, rel-err budget 2e-2):
#   1. f = relu(w1s^T @ x + b1f)            TensorE + ScalarE   [32, 2240]
#   2. per offset o: bc_o = W2BC_o @ f      TensorE (K=32)      [128, 2240] PSUM
#      (W2BC_o = w2 row o replicated across 128 output columns, so the
#       per-pixel kernel value arrives already broadcast over channels;
#       kerm itself is never materialized)
#      bcs_o = bc_o + b2[o]                 ScalarE PSUM->SBUF fp16
#      acc  += bcs_o * x_shift(o)           VectorE fp16 2x-mode (2 ops)
#
# Layout: x rows padded to stride 70 with 3 zero guard cols each side and
# 3 halo rows top/bottom -> every shifted read is exact zero padding (no
# mask). Accumulator kept in the same guarded layout so every DVE op is one
# contiguous stride-1 [128, 2240] access. A twin copy of x shifted by one
# element keeps odd-dj offsets 4-byte aligned (DVE 2x_1P mode requirement).
import numpy as np

EPS = 1e-5
KK = 7
C = 128
H = 64
W = 64
B = 4
HH = 32              # output rows per core
XROW = 70            # padded row stride: 3 | 64 | 3
NH = HH + 6          # rows incl. 3-row halos
XPAD = 4             # edge guard (even: preserves dj parity)
NXF = NH * XROW + 2 * XPAD   # 2668 x columns per core
QOFF = XPAD + 3 * XROW   # start of own rows in guarded coords (214)
NFREE = HH * XROW    # 2240: guarded output span
MMCH = 512           # matmul moving chunk (PSUM bank = 512 fp32)

_STATE = {}


def _build():
    import concourse.tile as tile
    from concourse import bacc, mybir

    f32 = mybir.dt.float32
    f16 = mybir.dt.float16
    nc = bacc.Bacc("TRN2", target_bir_lowering=False, debug=False)

    xa_d = nc.dram_tensor("xa", [C, NXF], f16, kind="ExternalInput").ap()
    xb_d = nc.dram_tensor("xb", [C, NXF], f16, kind="ExternalInput").ap()
    w1sT_d = nc.dram_tensor("w1sT", [C, 32], f16, kind="ExternalInput").ap()
    b1f_d = nc.dram_tensor("b1f", [32, 1], f32, kind="ExternalInput").ap()
    w2bc_d = nc.dram_tensor("w2bc", [32, 49 * C], f16, kind="ExternalInput").ap()
    b2bc_d = nc.dram_tensor("b2bc", [C, 49], f32, kind="ExternalInput").ap()
    out_d = nc.dram_tensor("out", [C, NFREE], f16, kind="ExternalOutput").ap()

    nmm = (NFREE + MMCH - 1) // MMCH  # 5 chunks: 512*4 + 192

    with tile.TileContext(nc) as tc:
        with (
            tc.tile_pool(name="consts", bufs=1) as cpool,
            tc.tile_pool(name="bcs", bufs=3) as spool,
            tc.tile_pool(name="prod", bufs=2) as ppool,
            tc.tile_pool(name="psum", bufs=1, space="PSUM") as pp,
        ):
            xa = cpool.tile([C, NXF], f16, tag="xa")
            nc.sync.dma_start(xa[:], xa_d)
            w1sT = cpool.tile([C, 32], f16, tag="w1")
            nc.sync.dma_start(w1sT[:], w1sT_d)
            b1f = cpool.tile([32, 1], f32, tag="b1")
            nc.sync.dma_start(b1f[:], b1f_d)
            xb = cpool.tile([C, NXF], f16, tag="xb")
            nc.sync.dma_start(xb[:], xb_d)
            w2bc = cpool.tile([32, 49 * C], f16, tag="w2bc")
            nc.sync.dma_start(w2bc[:], w2bc_d)
            b2bc = cpool.tile([C, 49], f32, tag="b2bc")
            nc.sync.dma_start(b2bc[:], b2bc_d)

            f_sb = cpool.tile([32, NFREE], f16, tag="f")
            acc = cpool.tile([C, NFREE], f16, tag="acc")

            # ---- kernel-feature generation: f = relu(w1s^T @ x + b1f) ----
            f_ps = pp.tile([32, NFREE], f32, tag="ps")
            for ci in range(nmm):
                c0 = ci * MMCH
                c1 = min(NFREE, c0 + MMCH)
                nc.tensor.matmul(
                    f_ps[:, c0:c1], w1sT[:], xa[:, QOFF + c0:QOFF + c1],
                    start=True, stop=True,
                )
            nc.scalar.activation(
                f_sb[:], f_ps[:], mybir.ActivationFunctionType.Relu, bias=b1f[:],
            )

            # ---- involution accumulate over the 49 offsets ----
            for o in range(49):
                ip, jp = divmod(o, 7)
                di, dj = ip - 3, jp - 3
                bc = pp.tile([C, NFREE], f32, tag="ps")
                for ci in range(nmm):
                    c0 = ci * MMCH
                    c1 = min(NFREE, c0 + MMCH)
                    nc.tensor.matmul(
                        bc[:, c0:c1], w2bc[:, o * C:(o + 1) * C], f_sb[:, c0:c1],
                        start=True, stop=True,
                    )
                bcs = spool.tile([C, NFREE], f16, tag="bcs")
                nc.scalar.activation(
                    bcs[:], bc[:], mybir.ActivationFunctionType.Identity,
                    bias=b2bc[:, o:o + 1],
                )
                base = QOFF + XROW * di + dj
                if base % 2 == 0:
                    xv = xa[:, base:base + NFREE]
                else:
                    xv = xb[:, base - 1:base - 1 + NFREE]
                if o == 0:
                    nc.vector.tensor_mul(acc[:], xv, bcs[:])
                else:
                    prod = ppool.tile([C, NFREE], f16, tag="prod")
                    nc.vector.tensor_mul(prod[:], xv, bcs[:])
                    nc.vector.tensor_add(acc[:], acc[:], prod[:])

            nc.sync.dma_start(out_d, acc[:])

    nc.compile()
    return nc


def _get_nc():
    if "nc" not in _STATE:
        _STATE["nc"] = _build()
    return _STATE["nc"]


def _host_prep(x, w1, b1, bn_gamma, bn_beta, bn_mean, bn_var, w2, b2):
    x = np.asarray(x, dtype=np.float32)
    scale = np.asarray(bn_gamma) / np.sqrt(np.asarray(bn_var) + EPS)
    w1s = (np.asarray(w1) * scale[:, None]).astype(np.float32)
    b1f = (np.asarray(b1) * scale + np.asarray(bn_beta)
           - np.asarray(bn_mean) * scale).astype(np.float32)
    w1sT = np.ascontiguousarray(w1s.T).astype(np.float16)        # [128, 32]
    b1fc = np.ascontiguousarray(b1f[:, None])                    # [32, 1]
    w2f = np.asarray(w2, np.float32)                             # [49, 32]
    # W2BC[r, o*128 + c] = w2[o, r]
    w2bc = np.ascontiguousarray(
        np.broadcast_to(w2f.T[:, :, None], (32, 49, C)).reshape(32, 49 * C)
    ).astype(np.float16)
    b2bc = np.ascontiguousarray(
        np.broadcast_to(np.asarray(b2, np.float32), (C, 49))
    )

    x16 = x.astype(np.float16)
    in_maps = []
    for core in range(8):
        b, half = divmod(core, 2)
        h0 = HH * half
        xa = np.zeros((C, NXF), dtype=np.float16)
        lo = max(0, h0 - 3)
        hi = min(H, h0 + HH + 3)
        body = xa[:, XPAD:XPAD + NH * XROW].reshape(C, NH, XROW)
        body[:, lo - (h0 - 3):hi - (h0 - 3), 3:3 + W] = x16[b, :, lo:hi, :]
        xbuf = np.zeros_like(xa)
        xbuf[:, :-1] = xa[:, 1:]
        in_maps.append({
            "xa": xa, "xb": xbuf, "w1sT": w1sT, "b1f": b1fc,
            "w2bc": w2bc, "b2bc": b2bc,
        })
    return in_maps


def run(inputs: dict, trace: bool = False):
    from concourse.bass_utils import run_bass_kernel_spmd

    nc = _get_nc()
    in_maps = _host_prep(**inputs)
    res = run_bass_kernel_spmd(
        nc, in_maps, core_ids=list(range(8)), trace=trace,
    )
    out = np.zeros((B, C, H, W), dtype=np.float32)
    for core in range(8):
        b, half = divmod(core, 2)
        h0 = HH * half
        o = res.results[core]["out"].reshape(C, HH, XROW)[:, :, 3:3 + W]
        out[b, :, h0:h0 + HH, :] = o.astype(np.float32)
    return out, res


def kernel(**inputs) -> np.ndarray:
    out, _ = run(inputs, trace=False)
    return out


# revision 6
# speedup vs baseline: 2.6118x; 1.6397x over previous
# Involution2d (K=7) Trainium2 kernel — 8-core SPMD, batch+spatial sharding.
#
# Sharding: 8 cores = (batch b in 0..3) x (H-half in 0..1). Each core computes
# a [128, 32, 64] output block.
#
# Per-core pipeline (fp16 data path, rel-err budget 2e-2):
#   1. f = relu(w1s^T @ xd + b1f)           TensorE + ScalarE   [32, 2048]
#   2. per offset o (49 total):
#      bc_o  = W2BC_o @ f                   TensorE (K=32)      [128, 2048] PSUM
#        (W2BC_o = w2 row o replicated into 128 columns -> the per-pixel
#         kernel value arrives already broadcast over channels; kerm is
#         never materialized and no per-offset DMA is needed)
#      bcs_o = bc_o + b2[o]                 ScalarE PSUM->SBUF fp16 (2 halves)
#      prod  = bcs_o * x_shift(o)           VectorE fp16 2x-mode
#      acc  += prod                         VectorE fp16 2x-mode
#
# x lives in a guarded stride-70 row layout (3 zero cols each side, 3 halo
# rows top/bottom) so every shifted read is exact zero padding — no mask.
# Shifted reads use 3D APs [128, 32, 64]; a twin copy of x offset by one
# element keeps odd-dj offsets 4-byte aligned (DVE 2x_1P mode requirement).
# acc/bcs/prod/f are dense [.., 2048] stride-1 tiles.
import numpy as np

EPS = 1e-5
KK = 7
C = 128
H = 64
W = 64
B = 4
HH = 32              # output rows per core
XROW = 70            # padded row stride: 3 | 64 | 3
NH = HH + 6          # rows incl. 3-row halos
XPAD = 4             # edge guard (even: preserves dj parity)
NXF = NH * XROW + 2 * XPAD   # 2668 x columns per core
QOFF = XPAD + 3 * XROW       # start of own rows in guarded coords (214)
P = HH * W           # 2048 dense output pixels
MMCH = 512           # matmul moving chunk (= PSUM bank, fp32)

_STATE = {}


def _build():
    import concourse.tile as tile
    from concourse import bacc, mybir

    f32 = mybir.dt.float32
    f16 = mybir.dt.float16
    nc = bacc.Bacc("TRN2", target_bir_lowering=False, debug=False)

    xa_d = nc.dram_tensor("xa", [C, NXF], f16, kind="ExternalInput").ap()
    xb_d = nc.dram_tensor("xb", [C, NXF], f16, kind="ExternalInput").ap()
    xd_d = nc.dram_tensor("xd", [C, P], f16, kind="ExternalInput").ap()
    w1sT_d = nc.dram_tensor("w1sT", [C, 32], f16, kind="ExternalInput").ap()
    b1f_d = nc.dram_tensor("b1f", [32, 1], f32, kind="ExternalInput").ap()
    w2bc_d = nc.dram_tensor("w2bc", [32, 49 * C], f16, kind="ExternalInput").ap()
    b2bc_d = nc.dram_tensor("b2bc", [C, 49], f32, kind="ExternalInput").ap()
    out_d = nc.dram_tensor("out", [C, P], f16, kind="ExternalOutput").ap()

    with tile.TileContext(nc) as tc:
        with (
            tc.tile_pool(name="consts", bufs=1) as cpool,
            tc.tile_pool(name="bcs", bufs=3) as spool,
            tc.tile_pool(name="prod", bufs=2) as ppool,
            tc.tile_pool(name="pgen", bufs=1, space="PSUM") as pgen,
            tc.tile_pool(name="pbc", bufs=2, space="PSUM") as pbc,
        ):
            xd = cpool.tile([C, P], f16, tag="xd")
            nc.sync.dma_start(xd[:], xd_d)
            w1sT = cpool.tile([C, 32], f16, tag="w1")
            nc.sync.dma_start(w1sT[:], w1sT_d)
            b1f = cpool.tile([32, 1], f32, tag="b1")
            nc.sync.dma_start(b1f[:], b1f_d)
            xa = cpool.tile([C, NXF], f16, tag="xa")
            nc.sync.dma_start(xa[:], xa_d)
            xb = cpool.tile([C, NXF], f16, tag="xb")
            nc.sync.dma_start(xb[:], xb_d)
            w2bc = cpool.tile([32, 49 * C], f16, tag="w2bc")
            nc.sync.dma_start(w2bc[:], w2bc_d)
            b2bc = cpool.tile([C, 49], f32, tag="b2bc")
            nc.sync.dma_start(b2bc[:], b2bc_d)

            f_sb = cpool.tile([32, P], f16, tag="f")
            acc = cpool.tile([C, P], f16, tag="acc")

            # guarded-layout shifted views of x (3D: [128, 32 rows, 64 w])
            xar = xa[:, XPAD:XPAD + NH * XROW].rearrange("p (h w) -> p h w", w=XROW)
            xbr = xb[:, XPAD:XPAD + NH * XROW].rearrange("p (h w) -> p h w", w=XROW)

            # ---- kernel-feature generation: f = relu(w1s^T @ xd + b1f) ----
            f_ps = pgen.tile([32, P], f32, tag="fps")
            for ci in range(P // MMCH):
                c0 = ci * MMCH
                nc.tensor.matmul(
                    f_ps[:, c0:c0 + MMCH], w1sT[:], xd[:, c0:c0 + MMCH],
                    start=True, stop=True,
                )
            nc.scalar.activation(
                f_sb[:], f_ps[:], mybir.ActivationFunctionType.Relu, bias=b1f[:],
            )

            # ---- involution accumulate over the 49 offsets ----
            HB = P // 2  # 1024: evac half (PSUM tile = 2 banks)
            for o in range(49):
                ip, jp = divmod(o, 7)
                di, dj = ip - 3, jp - 3
                bcs = spool.tile([C, P], f16, tag="bcs")
                for h2 in range(2):
                    bc = pbc.tile([C, HB], f32, tag="bc")
                    for ci in range(HB // MMCH):
                        c0 = h2 * HB + ci * MMCH
                        nc.tensor.matmul(
                            bc[:, ci * MMCH:(ci + 1) * MMCH],
                            w2bc[:, o * C:(o + 1) * C],
                            f_sb[:, c0:c0 + MMCH],
                            start=True, stop=True,
                        )
                    nc.scalar.activation(
                        bcs[:, h2 * HB:(h2 + 1) * HB], bc[:],
                        mybir.ActivationFunctionType.Identity,
                        bias=b2bc[:, o:o + 1],
                    )
                # shifted x view: rows di..di+32, cols 3+dj..67+dj of the
                # guarded layout; odd dj reads the 1-shifted twin for alignment
                r0 = 3 + di
                c0 = 3 + dj
                if c0 % 2 == 0:
                    xv = xar[:, r0:r0 + HH, c0:c0 + W]
                else:
                    xv = xbr[:, r0:r0 + HH, c0 - 1:c0 - 1 + W]
                bcsr = bcs.rearrange("p (h w) -> p h w", w=W)
                if o == 0:
                    accr = acc.rearrange("p (h w) -> p h w", w=W)
                    nc.vector.tensor_mul(accr, xv, bcsr)
                else:
                    prod = ppool.tile([C, P], f16, tag="prod")
                    prodr = prod.rearrange("p (h w) -> p h w", w=W)
                    nc.vector.tensor_mul(prodr, xv, bcsr)
                    nc.vector.tensor_add(acc[:], acc[:], prod[:])

            nc.sync.dma_start(out_d, acc[:])

    nc.compile()
    return nc


def _get_nc():
    if "nc" not in _STATE:
        _STATE["nc"] = _build()
    return _STATE["nc"]


def _host_prep(x, w1, b1, bn_gamma, bn_beta, bn_mean, bn_var, w2, b2):
    x = np.asarray(x, dtype=np.float32)
    scale = np.asarray(bn_gamma) / np.sqrt(np.asarray(bn_var) + EPS)
    w1s = (np.asarray(w1) * scale[:, None]).astype(np.float32)
    b1f = (np.asarray(b1) * scale + np.asarray(bn_beta)
           - np.asarray(bn_mean) * scale).astype(np.float32)
    w1sT = np.ascontiguousarray(w1s.T).astype(np.float16)        # [128, 32]
    b1fc = np.ascontiguousarray(b1f[:, None])                    # [32, 1]
    w2f = np.asarray(w2, np.float32)                             # [49, 32]
    # W2BC[r, o*128 + c] = w2[o, r]
    w2bc = np.ascontiguousarray(
        np.broadcast_to(w2f.T[:, :, None], (32, 49, C)).reshape(32, 49 * C)
    ).astype(np.float16)
    b2bc = np.ascontiguousarray(
        np.broadcast_to(np.asarray(b2, np.float32), (C, 49))
    )

    x16 = x.astype(np.float16)
    in_maps = []
    for core in range(8):
        b, half = divmod(core, 2)
        h0 = HH * half
        xa = np.zeros((C, NXF), dtype=np.float16)
        lo = max(0, h0 - 3)
        hi = min(H, h0 + HH + 3)
        body = xa[:, XPAD:XPAD + NH * XROW].reshape(C, NH, XROW)
        body[:, lo - (h0 - 3):hi - (h0 - 3), 3:3 + W] = x16[b, :, lo:hi, :]
        xbuf = np.zeros_like(xa)
        xbuf[:, :-1] = xa[:, 1:]
        xd = np.ascontiguousarray(x16[b, :, h0:h0 + HH, :].reshape(C, P))
        in_maps.append({
            "xa": xa, "xb": xbuf, "xd": xd, "w1sT": w1sT, "b1f": b1fc,
            "w2bc": w2bc, "b2bc": b2bc,
        })
    return in_maps


def run(inputs: dict, trace: bool = False):
    from concourse.bass_utils import run_bass_kernel_spmd

    nc = _get_nc()
    in_maps = _host_prep(**inputs)
    res = run_bass_kernel_spmd(
        nc, in_maps, core_ids=list(range(8)), trace=trace,
    )
    out = np.zeros((B, C, H, W), dtype=np.float32)
    for core in range(8):
        b, half = divmod(core, 2)
        h0 = HH * half
        o = res.results[core]["out"].reshape(C, HH, W)
        out[b, :, h0:h0 + HH, :] = o.astype(np.float32)
    return out, res


def kernel(**inputs) -> np.ndarray:
    out, _ = run(inputs, trace=False)
    return out


# revision 8
# speedup vs baseline: 2.6297x; 1.0069x over previous
# Involution2d (K=7) Trainium2 kernel — 8-core SPMD, batch+spatial sharding.
#
# Sharding: 8 cores = (batch b in 0..3) x (H-half in 0..1). Each core computes
# a [128, 32, 64] output block.
#
# Per-core pipeline (fp16 data path, rel-err budget 2e-2):
#   1. f = relu(w1s^T @ xd + b1f)           TensorE + ScalarE   [32, 2048]
#   2. per offset o (49 total):
#      bc_o  = W2BC_o @ f                   TensorE (K=32)      [128, 2048] PSUM
#        (W2BC_o = w2 row o replicated into 128 columns -> the per-pixel
#         kernel value arrives already broadcast over channels; kerm is
#         never materialized and no per-offset DMA is needed)
#      bcs_o = bc_o + b2[o]                 ScalarE PSUM->SBUF fp16 (2 halves)
#      prod  = bcs_o * x_shift(o)           VectorE fp16 2x-mode
#      acc  += prod                         VectorE fp16 2x-mode
#
# x lives in a guarded stride-70 row layout (3 zero cols each side, 3 halo
# rows top/bottom) so every shifted read is exact zero padding — no mask.
# Shifted reads use 3D APs [128, 32, 64]; a twin copy of x offset by one
# element keeps odd-dj offsets 4-byte aligned (DVE 2x_1P mode requirement).
# acc/bcs/prod/f are dense [.., 2048] stride-1 tiles.
import numpy as np

EPS = 1e-5
KK = 7
C = 128
H = 64
W = 64
B = 4
HH = 32              # output rows per core
XROW = 70            # padded row stride: 3 | 64 | 3
NH = HH + 6          # rows incl. 3-row halos
XPAD = 4             # edge guard (even: preserves dj parity)
NXF = NH * XROW + 2 * XPAD   # 2668 x columns per core
QOFF = XPAD + 3 * XROW       # start of own rows in guarded coords (214)
P = HH * W           # 2048 dense output pixels
MMCH = 512           # matmul moving chunk (= PSUM bank, fp32)

_STATE = {}


def _build():
    import concourse.tile as tile
    from concourse import bacc, mybir

    f32 = mybir.dt.float32
    f16 = mybir.dt.float16
    nc = bacc.Bacc("TRN2", target_bir_lowering=False, debug=False)

    xa_d = nc.dram_tensor("xa", [C, NXF], f16, kind="ExternalInput").ap()
    xb_d = nc.dram_tensor("xb", [C, NXF], f16, kind="ExternalInput").ap()
    xd_d = nc.dram_tensor("xd", [C, P], f16, kind="ExternalInput").ap()
    w1sT_d = nc.dram_tensor("w1sT", [C, 32], f16, kind="ExternalInput").ap()
    b1f_d = nc.dram_tensor("b1f", [32, 1], f32, kind="ExternalInput").ap()
    w2bc_d = nc.dram_tensor("w2bc", [32, 49 * C], f16, kind="ExternalInput").ap()
    b2bc_d = nc.dram_tensor("b2bc", [C, 49], f32, kind="ExternalInput").ap()
    out_d = nc.dram_tensor("out", [C, P], f16, kind="ExternalOutput").ap()

    with tile.TileContext(nc) as tc:
        with (
            tc.tile_pool(name="consts", bufs=1) as cpool,
            tc.tile_pool(name="bcs", bufs=3) as spool,
            tc.tile_pool(name="prod", bufs=2) as ppool,
            tc.tile_pool(name="pgen", bufs=1, space="PSUM") as pgen,
            tc.tile_pool(name="pbc", bufs=2, space="PSUM") as pbc,
        ):
            # spread input DMAs across engines -> parallel hardware queues
            xd = cpool.tile([C, P], f16, tag="xd")
            nc.sync.dma_start(xd[:], xd_d)
            w1sT = cpool.tile([C, 32], f16, tag="w1")
            nc.sync.dma_start(w1sT[:], w1sT_d)
            b1f = cpool.tile([32, 1], f32, tag="b1")
            nc.sync.dma_start(b1f[:], b1f_d)
            w2bc = cpool.tile([32, 49 * C], f16, tag="w2bc")
            nc.scalar.dma_start(w2bc[:, :49 * C // 2], w2bc_d[:, :49 * C // 2])
            nc.gpsimd.dma_start(w2bc[:, 49 * C // 2:], w2bc_d[:, 49 * C // 2:])
            b2bc = cpool.tile([C, 49], f32, tag="b2bc")
            nc.scalar.dma_start(b2bc[:], b2bc_d)
            xa = cpool.tile([C, NXF], f16, tag="xa")
            nc.scalar.dma_start(xa[:], xa_d)
            xb = cpool.tile([C, NXF], f16, tag="xb")
            nc.gpsimd.dma_start(xb[:], xb_d)

            f_sb = cpool.tile([32, P], f16, tag="f")
            acc = cpool.tile([C, P], f16, tag="acc")

            # guarded-layout shifted views of x (3D: [128, 32 rows, 64 w])
            xar = xa[:, XPAD:XPAD + NH * XROW].rearrange("p (h w) -> p h w", w=XROW)
            xbr = xb[:, XPAD:XPAD + NH * XROW].rearrange("p (h w) -> p h w", w=XROW)

            # ---- kernel-feature generation: f = relu(w1s^T @ xd + b1f) ----
            f_ps = pgen.tile([32, P], f32, tag="fps")
            for ci in range(P // MMCH):
                c0 = ci * MMCH
                nc.tensor.matmul(
                    f_ps[:, c0:c0 + MMCH], w1sT[:], xd[:, c0:c0 + MMCH],
                    start=True, stop=True,
                )
            nc.scalar.activation(
                f_sb[:], f_ps[:], mybir.ActivationFunctionType.Relu, bias=b1f[:],
            )

            # ---- involution accumulate over the 49 offsets ----
            HB = P // 2  # 1024: evac half (PSUM tile = 2 banks)
            for o in range(49):
                ip, jp = divmod(o, 7)
                di, dj = ip - 3, jp - 3
                bcs = spool.tile([C, P], f16, tag="bcs")
                for h2 in range(2):
                    bc = pbc.tile([C, HB], f32, tag="bc")
                    for ci in range(HB // MMCH):
                        c0 = h2 * HB + ci * MMCH
                        nc.tensor.matmul(
                            bc[:, ci * MMCH:(ci + 1) * MMCH],
                            w2bc[:, o * C:(o + 1) * C],
                            f_sb[:, c0:c0 + MMCH],
                            start=True, stop=True,
                        )
                    nc.scalar.activation(
                        bcs[:, h2 * HB:(h2 + 1) * HB], bc[:],
                        mybir.ActivationFunctionType.Identity,
                        bias=b2bc[:, o:o + 1],
                    )
                # shifted x view: rows di..di+32, cols 3+dj..67+dj of the
                # guarded layout; odd dj reads the 1-shifted twin for alignment
                r0 = 3 + di
                c0 = 3 + dj
                if c0 % 2 == 0:
                    xv = xar[:, r0:r0 + HH, c0:c0 + W]
                else:
                    xv = xbr[:, r0:r0 + HH, c0 - 1:c0 - 1 + W]
                bcsr = bcs.rearrange("p (h w) -> p h w", w=W)
                if o == 0:
                    accr = acc.rearrange("p (h w) -> p h w", w=W)
                    nc.vector.tensor_mul(accr, xv, bcsr)
                else:
                    prod = ppool.tile([C, P], f16, tag="prod")
                    prodr = prod.rearrange("p (h w) -> p h w", w=W)
                    nc.vector.tensor_mul(prodr, xv, bcsr)
                    nc.vector.tensor_add(acc[:], acc[:], prod[:])

            nc.sync.dma_start(out_d, acc[:])

    nc.compile()
    return nc


def _get_nc():
    if "nc" not in _STATE:
        _STATE["nc"] = _build()
    return _STATE["nc"]


def _host_prep(x, w1, b1, bn_gamma, bn_beta, bn_mean, bn_var, w2, b2):
    x = np.asarray(x, dtype=np.float32)
    scale = np.asarray(bn_gamma) / np.sqrt(np.asarray(bn_var) + EPS)
    w1s = (np.asarray(w1) * scale[:, None]).astype(np.float32)
    b1f = (np.asarray(b1) * scale + np.asarray(bn_beta)
           - np.asarray(bn_mean) * scale).astype(np.float32)
    w1sT = np.ascontiguousarray(w1s.T).astype(np.float16)        # [128, 32]
    b1fc = np.ascontiguousarray(b1f[:, None])                    # [32, 1]
    w2f = np.asarray(w2, np.float32)                             # [49, 32]
    # W2BC[r, o*128 + c] = w2[o, r]
    w2bc = np.ascontiguousarray(
        np.broadcast_to(w2f.T[:, :, None], (32, 49, C)).reshape(32, 49 * C)
    ).astype(np.float16)
    b2bc = np.ascontiguousarray(
        np.broadcast_to(np.asarray(b2, np.float32), (C, 49))
    )

    x16 = x.astype(np.float16)
    in_maps = []
    for core in range(8):
        b, half = divmod(core, 2)
        h0 = HH * half
        xa = np.zeros((C, NXF), dtype=np.float16)
        lo = max(0, h0 - 3)
        hi = min(H, h0 + HH + 3)
        body = xa[:, XPAD:XPAD + NH * XROW].reshape(C, NH, XROW)
        body[:, lo - (h0 - 3):hi - (h0 - 3), 3:3 + W] = x16[b, :, lo:hi, :]
        xbuf = np.zeros_like(xa)
        xbuf[:, :-1] = xa[:, 1:]
        xd = np.ascontiguousarray(x16[b, :, h0:h0 + HH, :].reshape(C, P))
        in_maps.append({
            "xa": xa, "xb": xbuf, "xd": xd, "w1sT": w1sT, "b1f": b1fc,
            "w2bc": w2bc, "b2bc": b2bc,
        })
    return in_maps


def run(inputs: dict, trace: bool = False):
    from concourse.bass_utils import run_bass_kernel_spmd

    nc = _get_nc()
    in_maps = _host_prep(**inputs)
    res = run_bass_kernel_spmd(
        nc, in_maps, core_ids=list(range(8)), trace=trace,
    )
    out = np.zeros((B, C, H, W), dtype=np.float32)
    for core in range(8):
        b, half = divmod(core, 2)
        h0 = HH * half
        o = res.results[core]["out"].reshape(C, HH, W)
        out[b, :, h0:h0 + HH, :] = o.astype(np.float32)
    return out, res


def kernel(**inputs) -> np.ndarray:
    out, _ = run(inputs, trace=False)
    return out
